# revision 11
# baseline (speedup 1.0000x reference)
"""MultiHeadEGATLayer on 8 Trainium2 cores.

Strategy (edge-parallel, dst-sharded):
- Nodes are split into 8 contiguous ranges of 6250; core c owns all edges whose
  dst falls in its range, sorted by dst and grouped into 128-node windows.
- Phase A (replicated on every core): node projection table
  T_src[n] = [ W_s.T nfeats[n] , W_nodes.T nfeats[n] + b ]  (256 f32) in DRAM.
- Phase B: per 128-edge tile, one indirect DMA gathers T_src[src[e]]; the edge
  projection (efeats @ W_e + b_e) and the dst projection (window-dense
  nfeats @ W_d expanded per-edge by a one-hot selector matmul) are accumulated
  in PSUM; leaky-relu -> attention scores -> exp (softmax without max-shift;
  scores are O(25) so fp32 exp is safe) -> exp-weighted messages and the
  per-window segment sums (numerator and denominator together) accumulate in
  PSUM via a second one-hot selector matmul.  h_out = numerator / denominator.
- Host only shards/sorts inputs and unpermutes outputs.
"""

import sys

sys.path.insert(0, "/opt/trn_rl_repo")

import numpy as np

N = 50000
E = 800000
DIN = 64
DOUT = 32
H = 4
NEG = 0.01
NCORES = 8
NPC = N // NCORES          # nodes per core
WSZ = 128                  # window = 128 nodes
NW = (NPC + WSZ - 1) // WSZ  # 49 window slots per core
PAD_DL = 999.0

TRACE = False
F32R = True      # use float32r (tf32-like) matmuls; ~1.4e-4 rel err, ~4x faster PE
LAST_RESULT = None


def _host_prep(nfeats, efeats, src, dst):
    """Build per-core padded edge streams. Returns (T, slots, per_core dicts)."""
    src = np.asarray(src).astype(np.int64).ravel()
    dst = np.asarray(dst).astype(np.int64).ravel()

    cores = []
    counts = np.zeros((NCORES, NW), dtype=np.int64)
    for c in range(NCORES):
        sel = np.nonzero((dst >= c * NPC) & (dst < (c + 1) * NPC))[0]
        dl = dst[sel] - c * NPC
        order = np.argsort(dl, kind="stable")
        e_sorted = sel[order]
        dl_sorted = dl[order]
        w_of = dl_sorted // WSZ
        counts[c] = np.bincount(w_of, minlength=NW)
        cores.append((e_sorted, dl_sorted, w_of))

    tpw = np.maximum(1, (counts.max(axis=0) + 127) // 128)  # tiles per window slot
    slot_start = np.concatenate([[0], np.cumsum(tpw)])      # in tiles
    T = int(slot_start[-1])

    ef_aug_T = np.concatenate(
        [efeats.T, np.ones((1, E), np.float32)], axis=0
    )  # [65, E]

    per_core = []
    for c in range(NCORES):
        e_sorted, dl_sorted, w_of = cores[c]
        grp_start = np.concatenate([[0], np.cumsum(counts[c])])
        rank = np.arange(len(e_sorted)) - grp_start[w_of]
        pos = slot_start[w_of] * 128 + rank

        srcv = np.zeros(T * 128, np.int32)
        dlv = np.full(T * 128, PAD_DL, np.float32)
        eft = np.zeros((65, T * 128), np.float32)
        orig = np.full(T * 128, -1, np.int64)

        srcv[pos] = src[e_sorted].astype(np.int32)
        dlv[pos] = (dl_sorted % WSZ).astype(np.float32)
        eft[:, pos] = ef_aug_T[:, e_sorted]
        orig[pos] = e_sorted

        per_core.append(
            dict(
                srcv=srcv.reshape(T, 128).T.copy(),      # [128, T]
                dl_col=dlv.reshape(T, 128).T.copy(),     # [128, T]
                dl_row=dlv[None, :].copy(),              # [1, T*128]
                eft=eft,                                 # [65, T*128]
                orig=orig,
            )
        )
    return T, tpw, per_core


def kernel(nfeats, efeats, src, dst, W_nodes, b_nodes, W_edges, b_edges, W_attn):
    global LAST_RESULT
    import concourse.bacc as bacc
    import concourse.bass as bass
    import concourse.tile as tile
    import concourse.mybir as mybir
    from concourse.bass_utils import run_bass_kernel_spmd

    f32 = mybir.dt.float32
    i32 = mybir.dt.int32
    fmm = mybir.dt.float32r if F32R else f32

    nfeats = np.asarray(nfeats, np.float32)
    efeats = np.asarray(efeats, np.float32)
    W_nodes = np.asarray(W_nodes, np.float32)
    b_nodes = np.asarray(b_nodes, np.float32)
    W_edges = np.asarray(W_edges, np.float32)
    b_edges = np.asarray(b_edges, np.float32)
    W_attn = np.asarray(W_attn, np.float32)

    T, tpw, per_core = _host_prep(nfeats, efeats, src, dst)

    NPAD = ((N + 127) // 128) * 128 + 128  # padded table rows (50048)
    NPC_PAD = NW * 128                     # padded per-core node rows (6272)

    # weights
    W_s = W_edges[0:DIN]            # [64, 128]
    W_e = W_edges[DIN : 2 * DIN]
    W_d = W_edges[2 * DIN :]
    W_e_aug = np.concatenate([W_e, b_edges[None, :]], axis=0)       # [65, 128]
    W_e_aug = np.concatenate([W_e_aug, np.zeros((65, 128), np.float32)], axis=1)
    W_d_aug = np.concatenate([W_d, np.zeros((1, H * DOUT), np.float32)], axis=0)
    W_d_aug = np.concatenate([W_d_aug, np.zeros((65, 128), np.float32)], axis=1)
    W_sn = np.concatenate([W_s, W_nodes], axis=1)                   # [64, 256]
    b_sn = np.concatenate([np.zeros(H * DOUT, np.float32), b_nodes])
    W_sn_aug = np.concatenate([W_sn, b_sn[None, :]], axis=0)        # [65, 256]
    wsum = W_attn.sum(axis=1)                                       # [32]
    wsum_repl = np.tile(np.tile(wsum, H)[None, :], (128, 1)).astype(np.float32)
    iota_col = np.arange(128, dtype=np.float32)[:, None]
    iota_repl = np.tile(np.arange(128, dtype=np.float32)[None, :], (128, 1))

    nf_aug_T = np.concatenate([nfeats.T, np.ones((1, N), np.float32)], axis=0)
    nfT_full = np.zeros((65, NPAD), np.float32)
    nfT_full[:, :N] = nf_aug_T

    # ---------------- build program ----------------
    nc = bacc.Bacc("TRN2", target_bir_lowering=False, debug=False,
                   num_devices=NCORES)

    d_nft_full = nc.dram_tensor("nft_full", [65, NPAD], fmm, kind="ExternalInput")
    d_nft_core = nc.dram_tensor("nft_core", [65, NPC_PAD], fmm, kind="ExternalInput")
    d_eft = nc.dram_tensor("eft", [65, T * 128], fmm, kind="ExternalInput")
    d_srcg = nc.dram_tensor("srcg", [128, T], i32, kind="ExternalInput")
    d_dlcol = nc.dram_tensor("dlcol", [128, T], f32, kind="ExternalInput")
    d_dlrow = nc.dram_tensor("dlrow", [1, T * 128], f32, kind="ExternalInput")
    d_we = nc.dram_tensor("we", [65, 256], fmm, kind="ExternalInput")
    d_wd = nc.dram_tensor("wd", [65, 256], fmm, kind="ExternalInput")
    d_wsn = nc.dram_tensor("wsn", [65, 256], fmm, kind="ExternalInput")
    d_iotac = nc.dram_tensor("iotac", [128, 1], f32, kind="ExternalInput")
    d_iotar = nc.dram_tensor("iotar", [128, 128], f32, kind="ExternalInput")
    d_wsr = nc.dram_tensor("wsr", [128, 128], f32, kind="ExternalInput")

    d_tsrc = nc.dram_tensor("tsrc", [NPAD, 256], fmm, kind="Internal")
    TCH = (T + 7) // 8  # f_out chunks of 8 tiles
    d_fout = nc.dram_tensor("fout", [TCH, 128, 8 * 128], f32, kind="ExternalOutput")
    d_hout = nc.dram_tensor("hout", [NW, 128, 128], f32, kind="ExternalOutput")

    NB = NPAD // 128  # table tiles (391)
    ACHUNK = 16       # phase A tiles per nft chunk
    FCH = 8           # f_out staging tiles per chunk

    with tile.TileContext(nc) as tc:
        with (
            tc.tile_pool(name="cst", bufs=1) as cst,
            tc.tile_pool(name="achunk", bufs=3) as achunk,
            tc.tile_pool(name="astage", bufs=3) as astage,
            tc.tile_pool(name="apsum", bufs=2, space="PSUM") as apsum,
            tc.tile_pool(name="gpool", bufs=18) as gpool,
            tc.tile_pool(name="maskp", bufs=10) as maskp,
            tc.tile_pool(name="work", bufs=4) as work,
            tc.tile_pool(name="wind", bufs=2) as wind,
            tc.tile_pool(name="fstage", bufs=2) as fstage,
            tc.tile_pool(name="bpsum", bufs=2, space="PSUM") as bpsum,
            tc.tile_pool(name="hpsum", bufs=2, space="PSUM") as hpsum,
        ):
            # constants
            wsn_sb = cst.tile([65, 256], fmm)
            nc.sync.dma_start(wsn_sb[:], d_wsn[:])
            we_sb = cst.tile([65, 256], fmm)
            nc.sync.dma_start(we_sb[:], d_we[:])
            wd_sb = cst.tile([65, 256], fmm)
            nc.sync.dma_start(wd_sb[:], d_wd[:])
            iotac_sb = cst.tile([128, 1], f32)
            nc.sync.dma_start(iotac_sb[:], d_iotac[:])
            iotar_sb = cst.tile([128, 128], f32)
            nc.sync.dma_start(iotar_sb[:], d_iotar[:])
            wsr_sb = cst.tile([128, 128], f32)
            nc.sync.dma_start(wsr_sb[:], d_wsr[:])
            srcg_sb = cst.tile([128, T], i32)
            nc.sync.dma_start(srcg_sb[:], d_srcg[:])
            dlcol_sb = cst.tile([128, T], f32)
            nc.sync.dma_start(dlcol_sb[:], d_dlcol[:])

            # ---------- Phase A: T_src table ----------
            AW = 4  # table tiles per write batch
            for a0 in range(0, NB, ACHUNK):
                an = min(ACHUNK, NB - a0)
                nch = achunk.tile([65, ACHUNK * 128], fmm, tag="nch")
                nc.sync.dma_start(nch[:, : an * 128],
                                  d_nft_full[:, a0 * 128 : (a0 + an) * 128])
                for k0 in range(0, an, AW):
                    kn = min(AW, an - k0)
                    st = astage.tile([128, AW * 256], fmm, tag="ast")
                    for k in range(k0, k0 + kn):
                        ps = apsum.tile([128, 256], f32, tag="aps")
                        nc.tensor.matmul(ps[:], nch[:, k * 128 : (k + 1) * 128],
                                         wsn_sb[:], start=True, stop=True)
                        nc.vector.tensor_copy(
                            st[:, (k - k0) * 256 : (k - k0 + 1) * 256], ps[:])
                    nb = a0 + k0
                    nc.sync.dma_start(
                        d_tsrc[nb * 128 : (nb + kn) * 128, :].rearrange(
                            "(t p) j -> p t j", p=128),
                        st[:, : kn * 256].rearrange("p (t j) -> p t j", j=256))

            tc.strict_bb_all_engine_barrier()

            # ---------- Phase B: edge sweep ----------
            # tile metadata: (window, tt, ntile)
            tmeta = []
            for w in range(NW):
                nt = int(tpw[w])
                for tt in range(nt):
                    tmeta.append((w, tt, nt))
            assert len(tmeta) == T

            pd_wins = {}
            h_pss = {}
            G_tiles = {}
            mask_tiles = {}
            for ch0 in range(0, T, FCH):
                chn = min(FCH, T - ch0)
                efch = work.tile([65, FCH * 128], fmm, tag="efch")
                nc.sync.dma_start(efch[:, : chn * 128],
                                  d_eft[:, ch0 * 128 : (ch0 + chn) * 128])
                fst = fstage.tile([128, FCH * 128], f32, tag="fst")
                a4ch = work.tile([128, FCH * 4], f32, tag="a4ch")

                # ---- sub-pass 1: F, lrelu, scores ----
                for ci in range(chn):
                    t = ch0 + ci
                    w, tt, ntile = tmeta[t]
                    if tt == 0:
                        nfw = wind.tile([65, 128], fmm, tag="nfw")
                        nc.sync.dma_start(nfw[:],
                                          d_nft_core[:, w * 128 : (w + 1) * 128])
                        pd_ps = bpsum.tile([128, 256], f32, tag="pdps")
                        nc.tensor.matmul(pd_ps[:], nfw[:], wd_sb[:],
                                         start=True, stop=True)
                        pd_win = wind.tile([128, 256], fmm, tag="pdwin")
                        nc.vector.tensor_copy(pd_win[:], pd_ps[:])
                        pd_wins[w] = pd_win
                    pd_win = pd_wins[w]

                    G = gpool.tile([128, 256], fmm, tag="G")
                    nc.gpsimd.indirect_dma_start(
                        out=G[:], out_offset=None, in_=d_tsrc[:],
                        in_offset=bass.IndirectOffsetOnAxis(
                            ap=srcg_sb[:, t : t + 1], axis=0))
                    G_tiles[t] = G

                    dlb = work.tile([128, 128], f32, tag="dlb")
                    nc.sync.dma_start(
                        dlb[:],
                        d_dlrow[0:1, t * 128 : (t + 1) * 128].to_broadcast([128, 128]))
                    maskT = work.tile([128, 128], fmm, tag="maskT")
                    nc.vector.tensor_scalar(
                        out=maskT[:], in0=dlb[:], scalar1=iotac_sb[:, 0:1],
                        scalar2=None, op0=mybir.AluOpType.is_equal)
                    mask = maskp.tile([128, 128], fmm, tag="mask")
                    nc.vector.tensor_scalar(
                        out=mask[:], in0=iotar_sb[:], scalar1=dlcol_sb[:, t : t + 1],
                        scalar2=None, op0=mybir.AluOpType.is_equal)
                    mask_tiles[t] = mask

                    F_ps = bpsum.tile([128, 256], f32, tag="fps")
                    nc.tensor.matmul(F_ps[:], efch[:, ci * 128 : (ci + 1) * 128],
                                     we_sb[:], start=True, stop=False)
                    nc.tensor.matmul(F_ps[:], maskT[:], pd_win[:],
                                     start=False, stop=True)
                    F_sb = work.tile([128, 128], f32, tag="fsb")
                    nc.vector.tensor_add(F_sb[:], F_ps[:, 0:128],
                                         G[:, 0:128].bitcast(f32))
                    nc.scalar.activation(
                        fst[:, ci * 128 : (ci + 1) * 128], F_sb[:],
                        mybir.ActivationFunctionType.Lrelu, alpha=NEG)

                    aw = work.tile([128, 128], f32, tag="aw")
                    nc.vector.tensor_tensor(
                        out=aw[:], in0=fst[:, ci * 128 : (ci + 1) * 128],
                        in1=wsr_sb[:], op=mybir.AluOpType.mult)
                    nc.vector.reduce_sum(
                        a4ch[:, ci * 4 : (ci + 1) * 4],
                        aw[:].rearrange("e (h f) -> e h f", h=H),
                        axis=mybir.AxisListType.X)

                nc.sync.dma_start(
                    d_fout[ch0 // FCH][:, : chn * 128], fst[:, : chn * 128])

                # ---- one exp per chunk (no ACT table thrash) ----
                each = work.tile([128, FCH * 4], fmm, tag="each")
                nc.scalar.activation(each[:, : chn * 4], a4ch[:, : chn * 4],
                                     mybir.ActivationFunctionType.Exp)

                # ---- sub-pass 2: messages + segment accumulate ----
                for ci in range(chn):
                    t = ch0 + ci
                    w, tt, ntile = tmeta[t]
                    if tt == 0:
                        h_pss[w] = hpsum.tile([128, 132], f32, tag="hps", name=f"hps_{w}")
                    h_ps = h_pss[w]
                    G = G_tiles.pop(t)
                    mask = mask_tiles.pop(t)

                    msg = work.tile([128, 132], fmm, tag="msg")
                    nc.scalar.copy(msg[:, 128:132], each[:, ci * 4 : (ci + 1) * 4])
                    nc.vector.tensor_tensor(
                        out=msg[:, 0:128].rearrange("e (h f) -> e h f", h=H),
                        in0=G[:, 128:256].rearrange("e (h f) -> e h f", h=H),
                        in1=each[:, ci * 4 : (ci + 1) * 4]
                        .rearrange("e (h one) -> e h one", one=1)
                        .to_broadcast([128, H, 32]),
                        op=mybir.AluOpType.mult)

                    nc.tensor.matmul(h_ps[:], mask[:], msg[:],
                                     start=(tt == 0), stop=(tt == ntile - 1))

                    if tt == ntile - 1:
                        hsb = wind.tile([128, 132], f32, tag="hsb")
                        nc.vector.tensor_copy(hsb[:], h_ps[:, 0:132])
                        den = wind.tile([128, 4], f32, tag="den")
                        nc.vector.tensor_scalar_add(den[:], hsb[:, 128:132], 1e-30)
                        rec = wind.tile([128, 4], f32, tag="rec")
                        nc.vector.reciprocal(rec[:], den[:])
                        how = wind.tile([128, 128], f32, tag="how")
                        for h in range(H):
                            nc.vector.tensor_scalar_mul(
                                how[:, h * 32 : (h + 1) * 32],
                                hsb[:, h * 32 : (h + 1) * 32], rec[:, h : h + 1])
                        nc.sync.dma_start(d_hout[w], how[:])
                        del h_pss[w], pd_wins[w]

    nc.compile()

    in_maps = []
    for c in range(NCORES):
        pc = per_core[c]
        nft_core = np.zeros((65, NPC_PAD), np.float32)
        nft_core[:, :NPC] = nf_aug_T[:, c * NPC : (c + 1) * NPC]
        in_maps.append({
            "nft_full": nfT_full, "nft_core": nft_core, "eft": pc["eft"],
            "srcg": pc["srcv"], "dlcol": pc["dl_col"], "dlrow": pc["dl_row"],
            "we": W_e_aug, "wd": W_d_aug, "wsn": W_sn_aug,
            "iotac": iota_col, "iotar": iota_repl, "wsr": wsum_repl,
        })

    res = run_bass_kernel_spmd(nc, in_maps, core_ids=list(range(NCORES)),
                               trace=TRACE)
    LAST_RESULT = res

    # ---------------- assemble outputs ----------------
    h_out = np.empty((N, H, DOUT), np.float32)
    f_out = np.empty((E, H * DOUT), np.float32)
    for c in range(NCORES):
        r = res.results[c]
        hout = r["hout"].reshape(NW * 128, 128)[:NPC]
        h_out[c * NPC : (c + 1) * NPC] = hout.reshape(NPC, H, DOUT)
        # fout dram: [TCH, 128e, 8t*128j] -> rows t*128+e
        TCH = (T + 7) // 8
        fout = (r["fout"].reshape(TCH, 128, 8, 128).transpose(0, 2, 1, 3)
                .reshape(TCH * 8 * 128, 128)[: T * 128])
        orig = per_core[c]["orig"]
        valid = orig >= 0
        f_out[orig[valid]] = fout[valid]
    return h_out, f_out.reshape(E, H, DOUT)


# revision 12
# speedup vs baseline: 1.1240x; 1.1240x over previous
"""MultiHeadEGATLayer on 8 Trainium2 cores.

Strategy (edge-parallel, dst-sharded):
- Nodes are split into 8 contiguous ranges of 6250; core c owns all edges whose
  dst falls in its range, sorted by dst and grouped into 128-node windows.
- Phase A (replicated on every core): node projection table
  T_src[n] = [ W_s.T nfeats[n] , W_nodes.T nfeats[n] + b ]  (256 f32) in DRAM.
- Phase B: per 128-edge tile, one indirect DMA gathers T_src[src[e]]; the edge
  projection (efeats @ W_e + b_e) and the dst projection (window-dense
  nfeats @ W_d expanded per-edge by a one-hot selector matmul) are accumulated
  in PSUM; leaky-relu -> attention scores -> exp (softmax without max-shift;
  scores are O(25) so fp32 exp is safe) -> exp-weighted messages and the
  per-window segment sums (numerator and denominator together) accumulate in
  PSUM via a second one-hot selector matmul.  h_out = numerator / denominator.
- Host only shards/sorts inputs and unpermutes outputs.
"""

import sys

sys.path.insert(0, "/opt/trn_rl_repo")

import numpy as np

N = 50000
E = 800000
DIN = 64
DOUT = 32
H = 4
NEG = 0.01
NCORES = 8
NPC = N // NCORES          # nodes per core
WSZ = 128                  # window = 128 nodes
NW = (NPC + WSZ - 1) // WSZ  # 49 window slots per core
PAD_DL = 999.0

TRACE = False
F32R = True      # use float32r (tf32-like) matmuls; ~1.4e-4 rel err, ~4x faster PE
LAST_RESULT = None


def _host_prep(nfeats, efeats, src, dst):
    """Build per-core padded edge streams. Returns (T, slots, per_core dicts)."""
    src = np.asarray(src).astype(np.int64).ravel()
    dst = np.asarray(dst).astype(np.int64).ravel()

    cores = []
    counts = np.zeros((NCORES, NW), dtype=np.int64)
    for c in range(NCORES):
        sel = np.nonzero((dst >= c * NPC) & (dst < (c + 1) * NPC))[0]
        dl = dst[sel] - c * NPC
        order = np.argsort(dl, kind="stable")
        e_sorted = sel[order]
        dl_sorted = dl[order]
        w_of = dl_sorted // WSZ
        counts[c] = np.bincount(w_of, minlength=NW)
        cores.append((e_sorted, dl_sorted, w_of))

    tpw = np.maximum(1, (counts.max(axis=0) + 127) // 128)  # tiles per window slot
    slot_start = np.concatenate([[0], np.cumsum(tpw)])      # in tiles
    T = int(slot_start[-1])

    ef_aug_T = np.concatenate(
        [efeats.T, np.ones((1, E), np.float32)], axis=0
    )  # [65, E]

    per_core = []
    for c in range(NCORES):
        e_sorted, dl_sorted, w_of = cores[c]
        grp_start = np.concatenate([[0], np.cumsum(counts[c])])
        rank = np.arange(len(e_sorted)) - grp_start[w_of]
        pos = slot_start[w_of] * 128 + rank

        srcv = np.zeros(T * 128, np.int32)
        dlv = np.full(T * 128, PAD_DL, np.float32)
        eft = np.zeros((65, T * 128), np.float32)
        orig = np.full(T * 128, -1, np.int64)

        srcv[pos] = src[e_sorted].astype(np.int32)
        dlv[pos] = (dl_sorted % WSZ).astype(np.float32)
        eft[:, pos] = ef_aug_T[:, e_sorted]
        orig[pos] = e_sorted

        per_core.append(
            dict(
                srcv=srcv.reshape(T, 128).T.copy(),      # [128, T]
                dl_col=dlv.reshape(T, 128).T.copy(),     # [128, T]
                dl_row=dlv[None, :].copy(),              # [1, T*128]
                eft=eft,                                 # [65, T*128]
                orig=orig,
            )
        )
    return T, tpw, per_core


def kernel(nfeats, efeats, src, dst, W_nodes, b_nodes, W_edges, b_edges, W_attn):
    global LAST_RESULT
    import concourse.bacc as bacc
    import concourse.bass as bass
    import concourse.tile as tile
    import concourse.mybir as mybir
    from concourse.bass_utils import run_bass_kernel_spmd

    f32 = mybir.dt.float32
    i32 = mybir.dt.int32
    fmm = mybir.dt.float32r if F32R else f32

    nfeats = np.asarray(nfeats, np.float32)
    efeats = np.asarray(efeats, np.float32)
    W_nodes = np.asarray(W_nodes, np.float32)
    b_nodes = np.asarray(b_nodes, np.float32)
    W_edges = np.asarray(W_edges, np.float32)
    b_edges = np.asarray(b_edges, np.float32)
    W_attn = np.asarray(W_attn, np.float32)

    T, tpw, per_core = _host_prep(nfeats, efeats, src, dst)

    NPAD = ((N + 127) // 128) * 128 + 128  # padded table rows (50048)
    NPC_PAD = NW * 128                     # padded per-core node rows (6272)

    # weights
    W_s = W_edges[0:DIN]            # [64, 128]
    W_e = W_edges[DIN : 2 * DIN]
    W_d = W_edges[2 * DIN :]
    W_e_aug = np.concatenate([W_e, b_edges[None, :]], axis=0)       # [65, 128]
    W_e_aug = np.concatenate([W_e_aug, np.zeros((65, 128), np.float32)], axis=1)
    W_d_aug = np.concatenate([W_d, np.zeros((1, H * DOUT), np.float32)], axis=0)
    W_d_aug = np.concatenate([W_d_aug, np.zeros((65, 128), np.float32)], axis=1)
    W_sn = np.concatenate([W_s, W_nodes], axis=1)                   # [64, 256]
    b_sn = np.concatenate([np.zeros(H * DOUT, np.float32), b_nodes])
    W_sn_aug = np.concatenate([W_sn, b_sn[None, :]], axis=0)        # [65, 256]
    wsum = W_attn.sum(axis=1)                                       # [32]
    wsum_repl = np.tile(np.tile(wsum, H)[None, :], (128, 1)).astype(np.float32)
    iota_col = np.arange(128, dtype=np.float32)[:, None]
    iota_repl = np.tile(np.arange(128, dtype=np.float32)[None, :], (128, 1))

    nf_aug_T = np.concatenate([nfeats.T, np.ones((1, N), np.float32)], axis=0)
    nfT_full = np.zeros((65, NPAD), np.float32)
    nfT_full[:, :N] = nf_aug_T

    # ---------------- build program ----------------
    nc = bacc.Bacc("TRN2", target_bir_lowering=False, debug=False,
                   num_devices=NCORES)

    d_nft_full = nc.dram_tensor("nft_full", [65, NPAD], fmm, kind="ExternalInput")
    d_nft_core = nc.dram_tensor("nft_core", [65, NPC_PAD], fmm, kind="ExternalInput")
    d_eft = nc.dram_tensor("eft", [65, T * 128], fmm, kind="ExternalInput")
    d_srcg = nc.dram_tensor("srcg", [128, T], i32, kind="ExternalInput")
    d_dlcol = nc.dram_tensor("dlcol", [128, T], f32, kind="ExternalInput")
    d_dlrow = nc.dram_tensor("dlrow", [1, T * 128], f32, kind="ExternalInput")
    d_we = nc.dram_tensor("we", [65, 256], fmm, kind="ExternalInput")
    d_wd = nc.dram_tensor("wd", [65, 256], fmm, kind="ExternalInput")
    d_wsn = nc.dram_tensor("wsn", [65, 256], fmm, kind="ExternalInput")
    d_iotac = nc.dram_tensor("iotac", [128, 1], f32, kind="ExternalInput")
    d_iotar = nc.dram_tensor("iotar", [128, 128], f32, kind="ExternalInput")
    d_wsr = nc.dram_tensor("wsr", [128, 128], f32, kind="ExternalInput")

    d_tsrc = nc.dram_tensor("tsrc", [NPAD, 256], fmm, kind="Internal")
    TCH = (T + 7) // 8  # f_out chunks of 8 tiles
    d_fout = nc.dram_tensor("fout", [TCH, 128, 8 * 128], f32, kind="ExternalOutput")
    d_hout = nc.dram_tensor("hout", [NW, 128, 128], f32, kind="ExternalOutput")

    NB = NPAD // 128  # table tiles (391)
    ACHUNK = 16       # phase A tiles per nft chunk
    FCH = 8           # f_out staging tiles per chunk

    with tile.TileContext(nc) as tc:
        with (
            tc.tile_pool(name="cst", bufs=1) as cst,
            tc.tile_pool(name="achunk", bufs=3) as achunk,
            tc.tile_pool(name="astage", bufs=3) as astage,
            tc.tile_pool(name="apsum", bufs=2, space="PSUM") as apsum,
            tc.tile_pool(name="psB", bufs=1, space="PSUM") as psB,
            tc.tile_pool(name="gpool", bufs=18) as gpool,
            tc.tile_pool(name="maskp", bufs=10) as maskp,
            tc.tile_pool(name="work", bufs=6) as work,
            tc.tile_pool(name="wind", bufs=2) as wind,
            tc.tile_pool(name="fstage", bufs=2) as fstage,
        ):
            # constants
            wsn_sb = cst.tile([65, 256], fmm)
            nc.sync.dma_start(wsn_sb[:], d_wsn[:])
            we_sb = cst.tile([65, 256], fmm)
            nc.sync.dma_start(we_sb[:], d_we[:])
            wd_sb = cst.tile([65, 256], fmm)
            nc.sync.dma_start(wd_sb[:], d_wd[:])
            iotac_sb = cst.tile([128, 1], f32)
            nc.sync.dma_start(iotac_sb[:], d_iotac[:])
            iotar_sb = cst.tile([128, 128], f32)
            nc.sync.dma_start(iotar_sb[:], d_iotar[:])
            wsr_sb = cst.tile([128, 128], f32)
            nc.sync.dma_start(wsr_sb[:], d_wsr[:])
            srcg_sb = cst.tile([128, T], i32)
            nc.sync.dma_start(srcg_sb[:], d_srcg[:])
            dlcol_sb = cst.tile([128, T], f32)
            nc.sync.dma_start(dlcol_sb[:], d_dlcol[:])

            # ---------- Phase A: T_src table ----------
            AW = 4  # table tiles per write batch
            for a0 in range(0, NB, ACHUNK):
                an = min(ACHUNK, NB - a0)
                nch = achunk.tile([65, ACHUNK * 128], fmm, tag="nch")
                nc.sync.dma_start(nch[:, : an * 128],
                                  d_nft_full[:, a0 * 128 : (a0 + an) * 128])
                for k0 in range(0, an, AW):
                    kn = min(AW, an - k0)
                    st = astage.tile([128, AW * 256], fmm, tag="ast")
                    for k in range(k0, k0 + kn):
                        ps = apsum.tile([128, 256], f32, tag="aps")
                        nc.tensor.matmul(ps[:], nch[:, k * 128 : (k + 1) * 128],
                                         wsn_sb[:], start=True, stop=True)
                        nc.vector.tensor_copy(
                            st[:, (k - k0) * 256 : (k - k0 + 1) * 256], ps[:])
                    nb = a0 + k0
                    nc.sync.dma_start(
                        d_tsrc[nb * 128 : (nb + kn) * 128, :].rearrange(
                            "(t p) j -> p t j", p=128),
                        st[:, : kn * 256].rearrange("p (t j) -> p t j", j=256))

            tc.strict_bb_all_engine_barrier()

            # ---------- Phase B: edge sweep ----------
            # tile metadata: (window, tt, ntile)
            tmeta = []
            for w in range(NW):
                nt = int(tpw[w])
                for tt in range(nt):
                    tmeta.append((w, tt, nt))
            assert len(tmeta) == T

            pd_wins = {}
            h_pss = {}
            G_tiles = {}
            mask_tiles = {}
            for ch0 in range(0, T, FCH):
                chn = min(FCH, T - ch0)
                efch = work.tile([65, FCH * 128], fmm, tag="efch")
                nc.sync.dma_start(efch[:, : chn * 128],
                                  d_eft[:, ch0 * 128 : (ch0 + chn) * 128])
                fst = fstage.tile([128, FCH * 128], f32, tag="fst")
                a4ch = work.tile([128, FCH * 4], f32, tag="a4ch")

                # ---- sub-pass 1: F, lrelu, scores ----
                for ci in range(chn):
                    t = ch0 + ci
                    w, tt, ntile = tmeta[t]
                    if tt == 0:
                        nfw = wind.tile([65, 128], fmm, tag="nfw")
                        nc.sync.dma_start(nfw[:],
                                          d_nft_core[:, w * 128 : (w + 1) * 128])
                        pd_ps = psB.tile([128, 256], f32, tag="pdps", bufs=1)
                        nc.tensor.matmul(pd_ps[:], nfw[:], wd_sb[:],
                                         start=True, stop=True)
                        pd_win = wind.tile([128, 256], fmm, tag="pdwin")
                        nc.vector.tensor_copy(pd_win[:], pd_ps[:])
                        pd_wins[w] = pd_win
                    pd_win = pd_wins[w]

                    G = gpool.tile([128, 256], fmm, tag="G")
                    nc.gpsimd.indirect_dma_start(
                        out=G[:], out_offset=None, in_=d_tsrc[:],
                        in_offset=bass.IndirectOffsetOnAxis(
                            ap=srcg_sb[:, t : t + 1], axis=0))
                    G_tiles[t] = G

                    dlb = work.tile([128, 128], f32, tag="dlb")
                    nc.sync.dma_start(
                        dlb[:],
                        d_dlrow[0:1, t * 128 : (t + 1) * 128].to_broadcast([128, 128]))
                    maskT = work.tile([128, 128], fmm, tag="maskT")
                    nc.vector.tensor_scalar(
                        out=maskT[:], in0=dlb[:], scalar1=iotac_sb[:, 0:1],
                        scalar2=None, op0=mybir.AluOpType.is_equal)
                    mask = maskp.tile([128, 128], fmm, tag="mask")
                    nc.vector.tensor_scalar(
                        out=mask[:], in0=iotar_sb[:], scalar1=dlcol_sb[:, t : t + 1],
                        scalar2=None, op0=mybir.AluOpType.is_equal)
                    mask_tiles[t] = mask

                    F_ps = psB.tile([128, 256], f32, tag="fps", bufs=3)
                    nc.tensor.matmul(F_ps[:], efch[:, ci * 128 : (ci + 1) * 128],
                                     we_sb[:], start=True, stop=False)
                    nc.tensor.matmul(F_ps[:], maskT[:], pd_win[:],
                                     start=False, stop=True)
                    F_sb = work.tile([128, 128], f32, tag="fsb")
                    nc.vector.tensor_add(F_sb[:], F_ps[:, 0:128],
                                         G[:, 0:128].bitcast(f32))
                    nc.scalar.activation(
                        fst[:, ci * 128 : (ci + 1) * 128], F_sb[:],
                        mybir.ActivationFunctionType.Lrelu, alpha=NEG)

                    aw = work.tile([128, 128], f32, tag="aw")
                    nc.vector.tensor_tensor(
                        out=aw[:], in0=fst[:, ci * 128 : (ci + 1) * 128],
                        in1=wsr_sb[:], op=mybir.AluOpType.mult)
                    nc.vector.reduce_sum(
                        a4ch[:, ci * 4 : (ci + 1) * 4],
                        aw[:].rearrange("e (h f) -> e h f", h=H),
                        axis=mybir.AxisListType.X)

                nc.sync.dma_start(
                    d_fout[ch0 // FCH][:, : chn * 128], fst[:, : chn * 128])

                # ---- one exp per chunk (no ACT table thrash) ----
                each = work.tile([128, FCH * 4], fmm, tag="each")
                nc.scalar.activation(each[:, : chn * 4], a4ch[:, : chn * 4],
                                     mybir.ActivationFunctionType.Exp)

                # ---- sub-pass 2: messages + segment accumulate ----
                for ci in range(chn):
                    t = ch0 + ci
                    w, tt, ntile = tmeta[t]
                    if tt == 0:
                        h_pss[w] = psB.tile([128, 132], f32, tag="hps", bufs=2, name=f"hps_{w}")
                    h_ps = h_pss[w]
                    G = G_tiles.pop(t)
                    mask = mask_tiles.pop(t)

                    msg = work.tile([128, 132], fmm, tag="msg")
                    nc.scalar.copy(msg[:, 128:132], each[:, ci * 4 : (ci + 1) * 4])
                    nc.vector.tensor_tensor(
                        out=msg[:, 0:128].rearrange("e (h f) -> e h f", h=H),
                        in0=G[:, 128:256].rearrange("e (h f) -> e h f", h=H),
                        in1=each[:, ci * 4 : (ci + 1) * 4]
                        .rearrange("e (h one) -> e h one", one=1)
                        .to_broadcast([128, H, 32]),
                        op=mybir.AluOpType.mult)

                    nc.tensor.matmul(h_ps[:], mask[:], msg[:],
                                     start=(tt == 0), stop=(tt == ntile - 1))

                    if tt == ntile - 1:
                        hsb = wind.tile([128, 132], f32, tag="hsb")
                        nc.vector.tensor_copy(hsb[:], h_ps[:, 0:132])
                        den = wind.tile([128, 4], f32, tag="den")
                        nc.vector.tensor_scalar_add(den[:], hsb[:, 128:132], 1e-30)
                        rec = wind.tile([128, 4], f32, tag="rec")
                        nc.vector.reciprocal(rec[:], den[:])
                        how = wind.tile([128, 128], f32, tag="how")
                        for h in range(H):
                            nc.vector.tensor_scalar_mul(
                                how[:, h * 32 : (h + 1) * 32],
                                hsb[:, h * 32 : (h + 1) * 32], rec[:, h : h + 1])
                        nc.sync.dma_start(d_hout[w], how[:])
                        del h_pss[w], pd_wins[w]

    nc.compile()

    in_maps = []
    for c in range(NCORES):
        pc = per_core[c]
        nft_core = np.zeros((65, NPC_PAD), np.float32)
        nft_core[:, :NPC] = nf_aug_T[:, c * NPC : (c + 1) * NPC]
        in_maps.append({
            "nft_full": nfT_full, "nft_core": nft_core, "eft": pc["eft"],
            "srcg": pc["srcv"], "dlcol": pc["dl_col"], "dlrow": pc["dl_row"],
            "we": W_e_aug, "wd": W_d_aug, "wsn": W_sn_aug,
            "iotac": iota_col, "iotar": iota_repl, "wsr": wsum_repl,
        })

    res = run_bass_kernel_spmd(nc, in_maps, core_ids=list(range(NCORES)),
                               trace=TRACE)
    LAST_RESULT = res

    # ---------------- assemble outputs ----------------
    h_out = np.empty((N, H, DOUT), np.float32)
    f_out = np.empty((E, H * DOUT), np.float32)
    for c in range(NCORES):
        r = res.results[c]
        hout = r["hout"].reshape(NW * 128, 128)[:NPC]
        h_out[c * NPC : (c + 1) * NPC] = hout.reshape(NPC, H, DOUT)
        # fout dram: [TCH, 128e, 8t*128j] -> rows t*128+e
        TCH = (T + 7) // 8
        fout = (r["fout"].reshape(TCH, 128, 8, 128).transpose(0, 2, 1, 3)
                .reshape(TCH * 8 * 128, 128)[: T * 128])
        orig = per_core[c]["orig"]
        valid = orig >= 0
        f_out[orig[valid]] = fout[valid]
    return h_out, f_out.reshape(E, H, DOUT)
